# revision 7
# baseline (speedup 1.0000x reference)
"""CTC prefix beam search decoder on Trainium2 (Bass/Tile).

Data-parallel: one utterance per NeuronCore (batch 8 across 8 cores).
Per core: T=128 sequential DP steps, beam width 8, 29 symbols, fully
unrolled. Merge detection uses exact trie-node ids (small ints in f32)
instead of the reference's uint32 rolling hash — provably the same merge
pattern absent hash collisions. XLA-CPU flush-to-zero is emulated at the
three product sites so the subnormal-band trajectory matches the
reference bit-for-bit.

Self-contained: hardcodes shapes [128, 8, 29] / lengths [8].
"""
import sys

import numpy as np

sys.path.insert(0, "/opt/trn_rl_repo")

from concourse import bacc, mybir  # noqa: E402
from concourse.bass import AP  # noqa: E402
from concourse.tile import TileContext  # noqa: E402
from concourse.bass_utils import run_bass_kernel_spmd  # noqa: E402

f32 = mybir.dt.float32
u32 = mybir.dt.uint32
ALU = mybir.AluOpType
AXL = mybir.AxisListType

T = 128
W = 8
S = 29
BLANK = 28
THRESH = 1e-3
FLT_MIN = 1.1754943508222875e-38
NCAND = W * (S + 1)  # 240, flat order j = w*30 + c' (c'=0 kept, c'=c+1 ext)

# STBIG column map. Pb/Pb_same share col 0 (Pb dies before Pb_same is
# written each step); Pnb/mPnb_kept share col 1.
C_PB = 0
C_PNB = 1
C_L = 2
C_P = 3
C_N = 4
C_LEN = 5
C_PREF = 6      # [., 6:134]
C_GEND = 134
C_TOT = 136
C_PLAST = 137
C_PNBS = 138
STW = 160


def _consts():
    c = {}
    c["c_ones8"] = np.ones((1, 8), np.float32)
    c["c_ones120"] = np.ones((1, 120), np.float32)
    c["c_ones128"] = np.ones((1, 128), np.float32)
    c["c_ones8col"] = np.ones((8, 1), np.float32)
    c["c_iota29b8"] = np.broadcast_to(np.arange(S, dtype=np.float32), (8, S)).copy()
    c["c_iota128p1b8"] = np.broadcast_to(
        np.arange(1, T + 1, dtype=np.float32), (8, T)).copy()
    c["c_iota128col"] = np.arange(T, dtype=np.float32).reshape(T, 1)
    c["c_iota129row"] = np.arange(T + 1, dtype=np.float32).reshape(1, T + 1)
    c["c_icol0"] = np.arange(120, dtype=np.float32).reshape(120, 1)
    c["c_icol1"] = np.arange(120, 240, dtype=np.float32).reshape(120, 1)
    j = np.arange(NCAND)
    par = j // 30
    isext = (j % 30 != 0).astype(np.float32)
    ec = (j % 30 - 1).astype(np.float32)
    p240t = (par[:, None] == np.arange(W)[None, :]).astype(np.float32)  # [240, 8]
    c["c_p240t0"] = p240t[:120].copy()
    c["c_p240t1"] = p240t[120:].copy()
    c["c_catt0"] = np.stack([isext[:120], ec[:120]], axis=1).astype(np.float32)
    c["c_catt1"] = np.stack([isext[120:], ec[120:]], axis=1).astype(np.float32)
    fresh = (8.0 + 8.0 * np.arange(T)[None, :] + np.arange(W)[:, None]).astype(
        np.float32)
    c["c_fresh"] = fresh  # [8, 128]
    c["c_m1col"] = np.full((8, 1), -1.0, np.float32)
    c["c_zero8"] = np.zeros((8, 1), np.float32)
    st = np.zeros((8, STW), np.float32)
    st[:, C_L] = -2.0
    st[0, C_L] = -1.0
    st[:, C_P] = -100.0
    st[:, C_N] = -(np.arange(W) + 2.0)
    st[0, C_N] = 0.0
    st[:, C_PREF:C_GEND] = -1.0
    st[0, C_PB] = 1.0
    c["c_stinit"] = st
    return c


_CONSTS = _consts()
_NC = None


def _build(t_steps=T):
    nc = bacc.Bacc("TRN2", target_bir_lowering=False, debug=False, num_devices=8)
    x_in = nc.declare_dram_parameter("x", [T, S], f32, isOutput=False)
    len_in = nc.declare_dram_parameter("lenf", [1, 1], f32, isOutput=False)
    cin = {}
    for k, v in _CONSTS.items():
        cin[k] = nc.declare_dram_parameter(k, list(v.shape), f32, isOutput=False)
    o_pref = nc.declare_dram_parameter("o_pref", [1, T], f32, isOutput=True)
    o_len = nc.declare_dram_parameter("o_len", [1, 1], f32, isOutput=True)
    o_bp = nc.declare_dram_parameter("o_bp", [1, 1], f32, isOutput=True)

    with TileContext(nc) as tc:
        with tc.tile_pool(name="sb", bufs=1) as pool, \
             tc.tile_pool(name="ps", bufs=1, space="PSUM") as psum:
            ct = {k: pool.tile(list(v.shape), f32, tag=k, name=k)
                  for k, v in _CONSTS.items()}
            stb = pool.tile([8, STW], f32, tag="stb")
            x128 = pool.tile([T, S], f32, tag="x128")
            lenf = pool.tile([1, 1], f32, tag="lenf")
            pgt = pool.tile([T, S], f32, tag="pgt")
            pma = pool.tile([T, S], f32, tag="pma")
            pmb_ = pool.tile([T, S], f32, tag="pmb_")
            tmaskf = pool.tile([T, 1], f32, tag="tmaskf")
            flatpm = pool.tile([1, T * S], f32, tag="flatpm")
            pmnb8 = pool.tile([8, T * S], f32, tag="pmnb8")
            pmb8 = pool.tile([8, T], f32, tag="pmb8")
            trace = pool.tile([1, T + 1], f32, tag="trace")
            sc30 = pool.tile([8, 30], f32, tag="sc30")
            flat = pool.tile([1, NCAND], f32, tag="flat")
            idx8 = pool.tile([1, 8], u32, tag="idx8")
            idxf = pool.tile([1, 8], f32, tag="idxf")
            gu = pool.tile([8, S], u32, tag="gu")
            mexu = pool.tile([8, S], u32, tag="mexu")
            base = pool.tile([8, S], f32, tag="base")
            plp = pool.tile([8, S], f32, tag="plp")
            ebuf = pool.tile([8, S], f32, tag="ebuf")
            fme = pool.tile([8, S], f32, tag="fme")
            fta = pool.tile([8, 1], f32, tag="fta")
            ftb = pool.tile([8, 1], f32, tag="ftb")
            fm1 = pool.tile([8, 1], f32, tag="fm1")
            fm2 = pool.tile([8, 1], f32, tag="fm2")
            # flip scratches: a/b: p|n columns->rows ([32,64], n in 2nd block);
            # c/d: E and its transpose; e/f: G and its transpose
            flipa = pool.tile([32, 64], f32, tag="flipa")
            flipb = pool.tile([32, 64], f32, tag="flipb")
            flipc = pool.tile([32, 32], f32, tag="flipc")
            flipd = pool.tile([32, 32], f32, tag="flipd")
            flipe = pool.tile([32, 32], f32, tag="flipe")
            flipf = pool.tile([32, 32], f32, tag="flipf")
            flipg = pool.tile([32, 32], f32, tag="flipg")
            fliph = pool.tile([32, 32], f32, tag="fliph")
            m1t = pool.tile([8, 8], f32, tag="m1t")
            m1 = pool.tile([8, 8], f32, tag="m1")
            mh = pool.tile([8, 8], f32, tag="mh")
            oh2s = pool.tile([8, 8], f32, tag="oh2s")
            oh0 = pool.tile([120, 8], f32, tag="oh0")
            oh1 = pool.tile([120, 8], f32, tag="oh1")
            isxu = pool.tile([8, 1], u32, tag="isxu")
            isxf = pool.tile([8, 1], f32, tag="isxf")
            wmasku = pool.tile([8, T], u32, tag="wmasku")
            lenb_s = pool.tile([T, 1], f32, tag="lenb_s")
            lmask = pool.tile([1, T + 1], f32, tag="lmask")
            bpp = pool.tile([1, T + 1], f32, tag="bpp")

            ps_a = psum.tile([8, 16], f32, tag="ps_a")     # PB8 | NB8
            ps_b = psum.tile([8, 40], f32, tag="ps_b")     # H 0:8 | ABS 8:9 | MEX 9:38
            ps_c = psum.tile([120, 8], f32, tag="ps_c")    # idx bcast
            ps_d = psum.tile([8, 152], f32, tag="ps_d")    # OH2 0:8|CATT 8:10|GATH 10:144
            ps_i = psum.tile([8, 464], f32, tag="ps_i")    # init builds
            ps_l = psum.tile([T, 1], f32, tag="ps_l")      # len bcast

            V, A, P, SY = nc.vector, nc.scalar, nc.tensor, nc.sync
            gf = flipe[0:8, 0:S]     # G lives in the flip scratch
            ehome = flipc[0:8, 0:S]  # E lives in the flip scratch

            # ---- init ----
            for k in _CONSTS:
                nc.gpsimd.dma_start(out=ct[k][:], in_=cin[k][:])
            nc.gpsimd.dma_start(out=stb[:], in_=cin["c_stinit"][:])
            nc.gpsimd.dma_start(out=x128[:], in_=x_in[:])
            nc.gpsimd.dma_start(out=lenf[:], in_=len_in[:])
            for f in (flipa, flipb, flipc, flipd, flipe, flipf, flipg, fliph):
                V.memset(f[:], 0.0)
            V.memset(trace[:], 0.0)

            P.matmul(ps_l[:], ct["c_ones128"][:], lenf[:], start=True, stop=True)
            A.copy(out=lenb_s[:], in_=ps_l[:])
            V.tensor_scalar(out=tmaskf[:], in0=ct["c_iota128col"][:],
                            scalar1=lenb_s[:], scalar2=None, op0=ALU.is_lt)
            V.tensor_scalar(out=pgt[:], in0=x128[:], scalar1=float(THRESH),
                            scalar2=None, op0=ALU.is_gt)
            V.tensor_tensor(out=pma[:], in0=x128[:], in1=pgt[:], op=ALU.mult)
            V.tensor_scalar(out=pmb_[:], in0=pma[:], scalar1=tmaskf[:],
                            scalar2=None, op0=ALU.mult)
            psrc = pmb_[:]
            src = AP(psrc.tensor, psrc.offset, [[psrc.ap[0][0], T], [1, S]])
            pdst = flatpm[:]
            dst = AP(pdst.tensor, pdst.offset, [[pdst.ap[0][0], 1], [1, T * S]])
            SY.dma_start(out=dst, in_=src)
            for i in range(8):
                sl = flatpm[0:1, i * 464:(i + 1) * 464]
                P.matmul(ps_i[:, 0:464], ct["c_ones8"][:], sl, start=True, stop=True)
                A.copy(out=pmnb8[:, i * 464:(i + 1) * 464], in_=ps_i[:, 0:464])
            fsrc = flatpm[:]
            blank_ap = AP(fsrc.tensor, fsrc.offset + BLANK, [[fsrc.ap[0][0], 1], [S, T]])
            P.matmul(ps_i[:, 0:T], ct["c_ones8"][:], blank_ap, start=True, stop=True)
            A.copy(out=pmb8[:], in_=ps_i[:, 0:T])
            p8 = pmnb8[:]
            nb_blank = AP(p8.tensor, p8.offset + BLANK, [[p8.ap[0][0], 8], [S, T]])
            V.memset(nb_blank, 0.0)

            # ---- DP loop ----
            for t in range(t_steps):
                pmnb_t = pmnb8[:, t * S:(t + 1) * S]
                # tot = Pb + Pnb
                V.tensor_tensor(out=stb[:, C_TOT:C_TOT + 1], in0=stb[:, C_PB:C_PB + 1],
                                in1=stb[:, C_PNB:C_PNB + 1], op=ALU.add)
                A.copy(out=trace[0:1, t:t + 1], in_=stb[0:1, C_TOT:C_TOT + 1])
                # G (into flipe) + u32 copy
                V.tensor_tensor(out=gf, in0=ct["c_iota29b8"][:],
                                in1=stb[:, C_L:C_L + 1].to_broadcast([8, S]),
                                op=ALU.is_equal)
                V.tensor_tensor(out=gu[:], in0=ct["c_iota29b8"][:],
                                in1=stb[:, C_L:C_L + 1].to_broadcast([8, S]),
                                op=ALU.is_equal)
                # base = select(G, Pb, tot)   (reads Pb before col0 is reused)
                V.tensor_copy(base[:], stb[:, C_TOT:C_TOT + 1].to_broadcast([8, S]))
                V.copy_predicated(base[:], gu[:],
                                  stb[:, C_PB:C_PB + 1].to_broadcast([8, S]))
                # p_last
                V.tensor_tensor(out=plp[:], in0=gf, in1=pmnb_t, op=ALU.mult)
                V.tensor_reduce(out=stb[:, C_PLAST:C_PLAST + 1], in_=plp[:],
                                axis=AXL.X, op=ALU.add)
                # Pnb_same = ftz(p_last * Pnb)   (reads Pnb before col1 reused)
                V.tensor_tensor(out=ftb[:], in0=stb[:, C_PLAST:C_PLAST + 1],
                                in1=stb[:, C_PNB:C_PNB + 1], op=ALU.mult)
                V.tensor_scalar(out=fm2[:], in0=ftb[:], scalar1=FLT_MIN,
                                scalar2=None, op0=ALU.is_ge)
                V.tensor_tensor(out=stb[:, C_PNBS:C_PNBS + 1], in0=ftb[:], in1=fm2[:],
                                op=ALU.mult)
                # Pb_same = ftz(pmb * tot) -> col0
                V.tensor_tensor(out=fta[:], in0=stb[:, C_TOT:C_TOT + 1],
                                in1=pmb8[:, t:t + 1], op=ALU.mult)
                V.tensor_scalar(out=fm1[:], in0=fta[:], scalar1=FLT_MIN,
                                scalar2=None, op0=ALU.is_ge)
                V.tensor_tensor(out=stb[:, C_PB:C_PB + 1], in0=fta[:], in1=fm1[:],
                                op=ALU.mult)
                # E = ftz(pmnb * base) -> flipc, then off-chain copy into stb
                V.tensor_tensor(out=ebuf[:], in0=pmnb_t, in1=base[:], op=ALU.mult)
                V.tensor_scalar(out=fme[:], in0=ebuf[:], scalar1=FLT_MIN,
                                scalar2=None, op0=ALU.is_ge)
                V.tensor_tensor(out=ehome, in0=ebuf[:], in1=fme[:], op=ALU.mult)
                # flip p|n to rows; broadcast; M1T/M1
                fa = flipa[:]
                pn_dst = AP(fa.tensor, fa.offset, [[fa.ap[0][0], 8], [32, 2]])
                V.tensor_copy(pn_dst, stb[:, C_P:C_P + 2])
                V.transpose(flipb[:], flipa[:])
                P.matmul(ps_a[:, 0:8], ct["c_ones8"][:], flipb[0:1, 0:8],
                         start=True, stop=True)
                P.matmul(ps_a[:, 8:16], ct["c_ones8"][:], flipb[0:1, 32:40],
                         start=True, stop=True)
                V.tensor_tensor(out=m1t[:], in0=ps_a[:, 0:8],
                                in1=stb[:, C_N:C_N + 1].to_broadcast([8, 8]),
                                op=ALU.is_equal)
                V.tensor_tensor(out=m1[:], in0=ps_a[:, 8:16],
                                in1=stb[:, C_P:C_P + 1].to_broadcast([8, 8]),
                                op=ALU.is_equal)
                # E^T, G^T
                V.transpose(flipd[:], flipc[:])
                V.transpose(flipf[:], flipe[:])
                # H, absorbed
                P.matmul(ps_b[:, 0:8], flipd[0:S, 0:8], flipf[0:S, 0:8],
                         start=True, stop=True)
                V.tensor_tensor(out=mh[:], in0=m1t[:], in1=ps_b[:, 0:8], op=ALU.mult)
                P.matmul(ps_b[:, 8:9], mh[:], ct["c_ones8col"][:],
                         start=True, stop=True)
                # mPnb_kept -> col1 ; sc_kept -> sc30 col0
                V.tensor_tensor(out=stb[:, C_PNB:C_PNB + 1],
                                in0=stb[:, C_PNBS:C_PNBS + 1], in1=ps_b[:, 8:9],
                                op=ALU.add)
                V.tensor_tensor(out=sc30[:, 0:1], in0=stb[:, C_PB:C_PB + 1],
                                in1=stb[:, C_PNB:C_PNB + 1], op=ALU.add)
                # MEX, sc_ext
                P.matmul(ps_b[:, 9:38], m1[:], gf, start=True, stop=True)
                V.tensor_scalar(out=mexu[:], in0=ps_b[:, 9:38], scalar1=0.0,
                                scalar2=None, op0=ALU.is_gt)
                A.copy(out=sc30[:, 1:30], in_=ehome)
                V.copy_predicated(sc30[:, 1:30], mexu[:],
                                  ct["c_m1col"][:].to_broadcast([8, S]))
                # flatten + top8
                s30 = sc30[:]
                fsrc30 = AP(s30.tensor, s30.offset, [[s30.ap[0][0], 8], [1, 30]])
                fl = flat[:]
                fdst30 = AP(fl.tensor, fl.offset, [[fl.ap[0][0], 1], [1, NCAND]])
                SY.dma_start(out=fdst30, in_=fsrc30)
                V.max(flipg[0:1, 0:8], flat[:])
                V.max_index(idx8[:], flipg[0:1, 0:8], flat[:])
                V.transpose(fliph[:], flipg[:])
                # onehot
                V.tensor_copy(idxf[:], idx8[:])
                P.matmul(ps_c[:], ct["c_ones120"][:], idxf[:], start=True, stop=True)
                V.tensor_tensor(out=oh0[:], in0=ps_c[:],
                                in1=ct["c_icol0"][:].to_broadcast([120, 8]),
                                op=ALU.is_equal)
                V.tensor_tensor(out=oh1[:], in0=ps_c[:],
                                in1=ct["c_icol1"][:].to_broadcast([120, 8]),
                                op=ALU.is_equal)
                # OH2, CATT, GATH
                P.matmul(ps_d[:, 0:8], ct["c_p240t0"][:], oh0[:], start=True, stop=False)
                P.matmul(ps_d[:, 0:8], ct["c_p240t1"][:], oh1[:], start=False, stop=True)
                V.tensor_copy(oh2s[:], ps_d[:, 0:8])
                P.matmul(ps_d[:, 8:10], oh0[:], ct["c_catt0"][:], start=True, stop=False)
                P.matmul(ps_d[:, 8:10], oh1[:], ct["c_catt1"][:], start=False, stop=True)
                P.matmul(ps_d[:, 10:10 + C_GEND], oh2s[:], stb[:, 0:C_GEND],
                         start=True, stop=True)
                g0 = 10
                # postprocess
                V.tensor_scalar(out=isxu[:], in0=ps_d[:, 8:9], scalar1=0.5,
                                scalar2=None, op0=ALU.is_gt)
                V.tensor_copy(isxf[:], ps_d[:, 8:9])
                V.tensor_copy(stb[:, 0:6], ps_d[:, g0:g0 + 6])
                V.copy_predicated(stb[:, C_L:C_L + 1], isxu[:], ps_d[:, 9:10])
                V.copy_predicated(stb[:, C_PB:C_PB + 1], isxu[:], ct["c_zero8"][:])
                V.copy_predicated(stb[:, C_P:C_P + 1], isxu[:],
                                  ps_d[:, g0 + C_N:g0 + C_N + 1])
                V.copy_predicated(stb[:, C_N:C_N + 1], isxu[:],
                                  ct["c_fresh"][:, t:t + 1])
                # newPnb (ext) = selected candidate's score = E value, from max
                V.copy_predicated(stb[:, C_PNB:C_PNB + 1], isxu[:],
                                  fliph[0:8, 0:1])
                # newlen
                V.tensor_tensor(out=stb[:, C_LEN:C_LEN + 1],
                                in0=ps_d[:, g0 + C_LEN:g0 + C_LEN + 1],
                                in1=isxf[:], op=ALU.add)
                # prefix: copy gathered, then write char where (pos+1==len)&ext
                A.copy(out=stb[:, C_PREF:C_GEND], in_=ps_d[:, g0 + C_PREF:g0 + C_GEND])
                V.tensor_scalar(out=wmasku[:], in0=ct["c_iota128p1b8"][:],
                                scalar1=stb[:, C_LEN:C_LEN + 1], scalar2=isxf[:],
                                op0=ALU.is_equal, op1=ALU.mult)
                V.copy_predicated(stb[:, C_PREF:C_GEND], wmasku[:],
                                  stb[:, C_L:C_L + 1].to_broadcast([8, T]))

            # ---- finalize ----
            V.tensor_tensor(out=stb[:, C_TOT:C_TOT + 1], in0=stb[:, C_PB:C_PB + 1],
                            in1=stb[:, C_PNB:C_PNB + 1], op=ALU.add)
            A.copy(out=trace[0:1, T:T + 1], in_=stb[0:1, C_TOT:C_TOT + 1])
            V.tensor_scalar(out=lmask[:], in0=ct["c_iota129row"][:],
                            scalar1=lenf[:], scalar2=None, op0=ALU.is_equal)
            V.tensor_tensor(out=bpp[:], in0=trace[:], in1=lmask[:], op=ALU.mult)
            V.tensor_reduce(out=bpp[0:1, 0:1], in_=bpp[:], axis=AXL.X, op=ALU.add)
            nc.gpsimd.dma_start(out=o_bp[:], in_=bpp[0:1, 0:1])
            nc.gpsimd.dma_start(out=o_pref[:], in_=stb[0:1, C_PREF:C_GEND])
            nc.gpsimd.dma_start(out=o_len[:], in_=stb[0:1, C_LEN:C_LEN + 1])

    nc.compile()
    return nc


def _get_nc():
    global _NC
    if _NC is None:
        _NC = _build(T)
    return _NC


def kernel(x, lengths):
    x = np.asarray(x, np.float32)
    lengths = np.asarray(lengths, np.int32)
    B = x.shape[1]
    nc = _get_nc()
    in_maps = []
    for b in range(B):
        m = {k: v for k, v in _CONSTS.items()}
        m["x"] = np.ascontiguousarray(x[:, b, :])
        m["lenf"] = np.array([[float(lengths[b])]], np.float32)
        in_maps.append(m)
    res = run_bass_kernel_spmd(nc, in_maps, list(range(B)))
    dec = np.zeros((B, T), np.int32)
    dl = np.zeros(B, np.int32)
    bp = np.zeros(B, np.float32)
    for b in range(B):
        r = res.results[b]
        dec[b] = np.rint(r["o_pref"][0]).astype(np.int32)
        dl[b] = np.int32(np.rint(r["o_len"][0, 0]))
        bp[b] = r["o_bp"][0, 0]
    return dec, dl, bp


# revision 11
# speedup vs baseline: 1.2982x; 1.2982x over previous
"""CTC prefix beam search decoder on Trainium2 (Bass/Tile).

Data-parallel: one utterance per NeuronCore (batch 8 across 8 cores).
Per core: T=128 sequential DP steps, beam width 8, 29 symbols, fully
unrolled. Merge detection uses exact trie-node ids (small ints in f32)
instead of the reference's uint32 rolling hash — provably the same merge
pattern absent hash collisions. XLA-CPU flush-to-zero is emulated at the
three product sites so the subnormal-band trajectory matches the
reference bit-for-bit.

Self-contained: hardcodes shapes [128, 8, 29] / lengths [8].
"""
import sys

import numpy as np

sys.path.insert(0, "/opt/trn_rl_repo")

from concourse import bacc, mybir  # noqa: E402
from concourse.bass import AP  # noqa: E402
from concourse.tile import TileContext  # noqa: E402
from concourse.bass_utils import run_bass_kernel_spmd  # noqa: E402

f32 = mybir.dt.float32
u32 = mybir.dt.uint32
ALU = mybir.AluOpType
AXL = mybir.AxisListType

T = 128
W = 8
S = 29
BLANK = 28
THRESH = 1e-3
FLT_MIN = 1.1754943508222875e-38
NCAND = W * (S + 1)  # 240, flat order j = w*30 + c' (c'=0 kept, c'=c+1 ext)

# STBIG column map. Pb/Pb_same share col 0 (Pb dies before Pb_same is
# written each step); Pnb/mPnb_kept share col 1.
C_PB = 0
C_PNB = 1
C_L = 2
C_P = 3
C_N = 4
C_LEN = 5
C_PREF = 6      # [., 6:134]
C_GEND = 134
C_TOT = 136
C_PLAST = 137
C_PNBS = 138
STW = 160


def _consts():
    c = {}
    c["c_ones8"] = np.ones((1, 8), np.float32)
    c["c_ones128"] = np.ones((1, 128), np.float32)
    c["c_ones8col"] = np.ones((8, 1), np.float32)
    c["c_iota29b8"] = np.broadcast_to(np.arange(S, dtype=np.float32), (8, S)).copy()
    c["c_iota128p1b8"] = np.broadcast_to(
        np.arange(1, T + 1, dtype=np.float32), (8, T)).copy()
    c["c_iota128col"] = np.arange(T, dtype=np.float32).reshape(T, 1)
    c["c_iota129row"] = np.arange(T + 1, dtype=np.float32).reshape(1, T + 1)
    c["c_i30col"] = (30.0 * np.arange(W, dtype=np.float32)).reshape(W, 1)
    c["c_thresh64"] = np.tile(30.0 * np.arange(1, 9, dtype=np.float32), 8).reshape(1, 64)
    fresh = (8.0 + 8.0 * np.arange(T)[None, :] + np.arange(W)[:, None]).astype(
        np.float32)
    c["c_fresh"] = fresh  # [8, 128]
    c["c_m1col"] = np.full((8, 1), -1.0, np.float32)
    c["c_zero8"] = np.zeros((8, 1), np.float32)
    st = np.zeros((8, STW), np.float32)
    st[:, C_L] = -2.0
    st[0, C_L] = -1.0
    st[:, C_P] = -100.0
    st[:, C_N] = -(np.arange(W) + 2.0)
    st[0, C_N] = 0.0
    st[:, C_PREF:C_GEND] = -1.0
    st[0, C_PB] = 1.0
    c["c_stinit"] = st
    return c


_CONSTS = _consts()
_NC = None


def _build(t_steps=T):
    nc = bacc.Bacc("TRN2", target_bir_lowering=False, debug=False, num_devices=8)
    x_in = nc.declare_dram_parameter("x", [T, S], f32, isOutput=False)
    len_in = nc.declare_dram_parameter("lenf", [1, 1], f32, isOutput=False)
    cin = {}
    for k, v in _CONSTS.items():
        cin[k] = nc.declare_dram_parameter(k, list(v.shape), f32, isOutput=False)
    o_pref = nc.declare_dram_parameter("o_pref", [1, T], f32, isOutput=True)
    o_len = nc.declare_dram_parameter("o_len", [1, 1], f32, isOutput=True)
    o_bp = nc.declare_dram_parameter("o_bp", [1, 1], f32, isOutput=True)

    with TileContext(nc) as tc:
        with tc.tile_pool(name="sb", bufs=1) as pool, \
             tc.tile_pool(name="ps", bufs=1, space="PSUM") as psum:
            ct = {k: pool.tile(list(v.shape), f32, tag=k, name=k)
                  for k, v in _CONSTS.items()}
            stb = pool.tile([8, STW], f32, tag="stb")
            x128 = pool.tile([T, S], f32, tag="x128")
            lenf = pool.tile([1, 1], f32, tag="lenf")
            pgt = pool.tile([T, S], f32, tag="pgt")
            pma = pool.tile([T, S], f32, tag="pma")
            pmb_ = pool.tile([T, S], f32, tag="pmb_")
            tmaskf = pool.tile([T, 1], f32, tag="tmaskf")
            flatpm = pool.tile([1, T * S], f32, tag="flatpm")
            pmnb8 = pool.tile([8, T * S], f32, tag="pmnb8")
            pmb8 = pool.tile([8, T], f32, tag="pmb8")
            trace = pool.tile([1, T + 1], f32, tag="trace")
            sc30 = pool.tile([8, 30], f32, tag="sc30")
            flat = pool.tile([1, NCAND], f32, tag="flat")
            idx8 = pool.tile([1, 8], u32, tag="idx8")
            idxf = pool.tile([1, 8], f32, tag="idxf")
            gu = pool.tile([8, S], u32, tag="gu")
            mexu = pool.tile([8, S], u32, tag="mexu")
            base = pool.tile([8, S], f32, tag="base")
            plp = pool.tile([8, S], f32, tag="plp")
            ebuf = pool.tile([8, S], f32, tag="ebuf")
            fme = pool.tile([8, S], f32, tag="fme")
            fta = pool.tile([8, 1], f32, tag="fta")
            ftb = pool.tile([8, 1], f32, tag="ftb")
            fm1 = pool.tile([8, 1], f32, tag="fm1")
            fm2 = pool.tile([8, 1], f32, tag="fm2")
            # flip scratches: a/b: p|n columns->rows ([32,64], n in 2nd block);
            # c/d: E and its transpose; e/f: G and its transpose
            flipa = pool.tile([32, 64], f32, tag="flipa")
            flipb = pool.tile([32, 64], f32, tag="flipb")
            flipc = pool.tile([32, 32], f32, tag="flipc")
            flipd = pool.tile([32, 32], f32, tag="flipd")
            flipe = pool.tile([32, 32], f32, tag="flipe")
            flipf = pool.tile([32, 32], f32, tag="flipf")
            flipg = pool.tile([32, 96], f32, tag="flipg")
            fliph = pool.tile([32, 96], f32, tag="fliph")
            m1t = pool.tile([8, 8], f32, tag="m1t")
            m1 = pool.tile([8, 8], f32, tag="m1")
            mh = pool.tile([8, 8], f32, tag="mh")
            oh2s = pool.tile([8, 8], f32, tag="oh2s")
            data3 = pool.tile([8, 3], f32, tag="data3")
            cmp64 = pool.tile([1, 64], f32, tag="cmp64")
            isxu = pool.tile([8, 1], u32, tag="isxu")
            isxf = pool.tile([8, 1], f32, tag="isxf")
            wmasku = pool.tile([8, T], u32, tag="wmasku")
            lenb_s = pool.tile([T, 1], f32, tag="lenb_s")
            lmask = pool.tile([1, T + 1], f32, tag="lmask")
            bpp = pool.tile([1, T + 1], f32, tag="bpp")

            ps_a = psum.tile([8, 24], f32, tag="ps_a")     # PB8 | NB8 | b30
            ps_b = psum.tile([8, 40], f32, tag="ps_b")     # H 0:8 | ABS 8:9 | MEX 9:38
            ps_d = psum.tile([8, 152], f32, tag="ps_d")    # OH2 0:8|CATT 8:10|GATH 10:144
            ps_i = psum.tile([8, 464], f32, tag="ps_i")    # init builds
            ps_l = psum.tile([T, 1], f32, tag="ps_l")      # len bcast

            V, A, P, SY = nc.vector, nc.scalar, nc.tensor, nc.sync
            gf = flipe[0:8, 0:S]     # G lives in the flip scratch
            ehome = flipc[0:8, 0:S]  # E lives in the flip scratch

            # ---- init ----
            for k in _CONSTS:
                nc.gpsimd.dma_start(out=ct[k][:], in_=cin[k][:])
            nc.gpsimd.dma_start(out=stb[:], in_=cin["c_stinit"][:])
            nc.gpsimd.dma_start(out=x128[:], in_=x_in[:])
            nc.gpsimd.dma_start(out=lenf[:], in_=len_in[:])
            for f in (flipa, flipb, flipc, flipd, flipe, flipf, flipg, fliph):
                V.memset(f[:], 0.0)
            V.memset(trace[:], 0.0)

            P.matmul(ps_l[:], ct["c_ones128"][:], lenf[:], start=True, stop=True)
            A.copy(out=lenb_s[:], in_=ps_l[:])
            V.tensor_scalar(out=tmaskf[:], in0=ct["c_iota128col"][:],
                            scalar1=lenb_s[:], scalar2=None, op0=ALU.is_lt)
            V.tensor_scalar(out=pgt[:], in0=x128[:], scalar1=float(THRESH),
                            scalar2=None, op0=ALU.is_gt)
            V.tensor_tensor(out=pma[:], in0=x128[:], in1=pgt[:], op=ALU.mult)
            V.tensor_scalar(out=pmb_[:], in0=pma[:], scalar1=tmaskf[:],
                            scalar2=None, op0=ALU.mult)
            psrc = pmb_[:]
            src = AP(psrc.tensor, psrc.offset, [[psrc.ap[0][0], T], [1, S]])
            pdst = flatpm[:]
            dst = AP(pdst.tensor, pdst.offset, [[pdst.ap[0][0], 1], [1, T * S]])
            SY.dma_start(out=dst, in_=src)
            for i in range(8):
                sl = flatpm[0:1, i * 464:(i + 1) * 464]
                P.matmul(ps_i[:, 0:464], ct["c_ones8"][:], sl, start=True, stop=True)
                A.copy(out=pmnb8[:, i * 464:(i + 1) * 464], in_=ps_i[:, 0:464])
            fsrc = flatpm[:]
            blank_ap = AP(fsrc.tensor, fsrc.offset + BLANK, [[fsrc.ap[0][0], 1], [S, T]])
            P.matmul(ps_i[:, 0:T], ct["c_ones8"][:], blank_ap, start=True, stop=True)
            A.copy(out=pmb8[:], in_=ps_i[:, 0:T])
            p8 = pmnb8[:]
            nb_blank = AP(p8.tensor, p8.offset + BLANK, [[p8.ap[0][0], 8], [S, T]])
            V.memset(nb_blank, 0.0)

            # ---- DP loop ----
            for t in range(t_steps):
                pmnb_t = pmnb8[:, t * S:(t + 1) * S]
                # MEX head first so PE/compare work overlaps the E chain
                fa = flipa[:]
                pn_dst = AP(fa.tensor, fa.offset, [[fa.ap[0][0], 8], [32, 2]])
                V.tensor_copy(pn_dst, stb[:, C_P:C_P + 2])
                V.transpose(flipb[:], flipa[:])
                P.matmul(ps_a[:, 0:8], ct["c_ones8"][:], flipb[0:1, 0:8],
                         start=True, stop=True)
                P.matmul(ps_a[:, 8:16], ct["c_ones8"][:], flipb[0:1, 32:40],
                         start=True, stop=True)
                V.tensor_tensor(out=m1t[:], in0=ps_a[:, 0:8],
                                in1=stb[:, C_N:C_N + 1].to_broadcast([8, 8]),
                                op=ALU.is_equal)
                V.tensor_tensor(out=m1[:], in0=ps_a[:, 8:16],
                                in1=stb[:, C_P:C_P + 1].to_broadcast([8, 8]),
                                op=ALU.is_equal)
                # tot = Pb + Pnb
                V.tensor_tensor(out=stb[:, C_TOT:C_TOT + 1], in0=stb[:, C_PB:C_PB + 1],
                                in1=stb[:, C_PNB:C_PNB + 1], op=ALU.add)
                A.copy(out=trace[0:1, t:t + 1], in_=stb[0:1, C_TOT:C_TOT + 1])
                # G
                V.tensor_tensor(out=gf, in0=ct["c_iota29b8"][:],
                                in1=stb[:, C_L:C_L + 1].to_broadcast([8, S]),
                                op=ALU.is_equal)
                V.tensor_tensor(out=gu[:], in0=ct["c_iota29b8"][:],
                                in1=stb[:, C_L:C_L + 1].to_broadcast([8, S]),
                                op=ALU.is_equal)
                # MEX early: only needs m1 + G, keeps PE ahead of the E chain
                P.matmul(ps_b[:, 9:38], m1[:], gf, start=True, stop=True)
                V.tensor_scalar(out=mexu[:], in0=ps_b[:, 9:38], scalar1=0.0,
                                scalar2=None, op0=ALU.is_gt)
                # base = select(G, Pb, tot)
                V.tensor_copy(base[:], stb[:, C_TOT:C_TOT + 1].to_broadcast([8, S]))
                V.copy_predicated(base[:], gu[:],
                                  stb[:, C_PB:C_PB + 1].to_broadcast([8, S]))
                # p_last
                V.tensor_tensor(out=plp[:], in0=gf, in1=pmnb_t, op=ALU.mult)
                V.tensor_reduce(out=stb[:, C_PLAST:C_PLAST + 1], in_=plp[:],
                                axis=AXL.X, op=ALU.add)
                # Pnb_same = ftz(p_last * Pnb)
                V.tensor_tensor(out=ftb[:], in0=stb[:, C_PLAST:C_PLAST + 1],
                                in1=stb[:, C_PNB:C_PNB + 1], op=ALU.mult)
                V.tensor_scalar(out=fm2[:], in0=ftb[:], scalar1=FLT_MIN,
                                scalar2=None, op0=ALU.is_ge)
                V.tensor_tensor(out=stb[:, C_PNBS:C_PNBS + 1], in0=ftb[:], in1=fm2[:],
                                op=ALU.mult)
                # Pb_same = ftz(pmb * tot) -> col0
                V.tensor_tensor(out=fta[:], in0=stb[:, C_TOT:C_TOT + 1],
                                in1=pmb8[:, t:t + 1], op=ALU.mult)
                V.tensor_scalar(out=fm1[:], in0=fta[:], scalar1=FLT_MIN,
                                scalar2=None, op0=ALU.is_ge)
                V.tensor_tensor(out=stb[:, C_PB:C_PB + 1], in0=fta[:], in1=fm1[:],
                                op=ALU.mult)
                # E = ftz(pmnb * base) -> flipc
                V.tensor_tensor(out=ebuf[:], in0=pmnb_t, in1=base[:], op=ALU.mult)
                V.tensor_scalar(out=fme[:], in0=ebuf[:], scalar1=FLT_MIN,
                                scalar2=None, op0=ALU.is_ge)
                V.tensor_tensor(out=ehome, in0=ebuf[:], in1=fme[:], op=ALU.mult)
                # E^T, G^T; H; absorbed
                V.transpose(flipd[:], flipc[:])
                V.transpose(flipf[:], flipe[:])
                P.matmul(ps_b[:, 0:8], flipd[0:S, 0:8], flipf[0:S, 0:8],
                         start=True, stop=True)
                V.tensor_tensor(out=mh[:], in0=m1t[:], in1=ps_b[:, 0:8], op=ALU.mult)
                P.matmul(ps_b[:, 8:9], mh[:], ct["c_ones8col"][:],
                         start=True, stop=True)
                # mPnb_kept -> col1 ; sc_kept -> sc30 col0
                V.tensor_tensor(out=stb[:, C_PNB:C_PNB + 1],
                                in0=stb[:, C_PNBS:C_PNBS + 1], in1=ps_b[:, 8:9],
                                op=ALU.add)
                V.tensor_tensor(out=sc30[:, 0:1], in0=stb[:, C_PB:C_PB + 1],
                                in1=stb[:, C_PNB:C_PNB + 1], op=ALU.add)
                A.copy(out=sc30[:, 1:30], in_=ehome)
                V.copy_predicated(sc30[:, 1:30], mexu[:],
                                  ct["c_m1col"][:].to_broadcast([8, S]))
                # flatten + top8
                s30 = sc30[:]
                fsrc30 = AP(s30.tensor, s30.offset, [[s30.ap[0][0], 8], [1, 30]])
                fl = flat[:]
                fdst30 = AP(fl.tensor, fl.offset, [[fl.ap[0][0], 1], [1, NCAND]])
                SY.dma_start(out=fdst30, in_=fsrc30)
                V.max(flipg[0:1, 0:8], flat[:])
                V.max_index(idx8[:], flipg[0:1, 0:8], flat[:])
                # decompose idx: w = sum_k (idx >= 30k); 30w ; c' = idx - 30w
                V.tensor_copy(idxf[:], idx8[:])
                ifl = idxf[:]
                idx_rep8 = AP(ifl.tensor, ifl.offset, [[ifl.ap[0][0], 1], [1, 8], [0, 8]])
                V.tensor_tensor(out=cmp64[:], in0=idx_rep8, in1=ct["c_thresh64"][:],
                                op=ALU.is_ge)
                c64 = cmp64[:]
                cmp3d = AP(c64.tensor, c64.offset, [[c64.ap[0][0], 1], [8, 8], [1, 8]])
                V.tensor_reduce(out=flipg[0:1, 48:56], in_=cmp3d, axis=AXL.X,
                                op=ALU.add)
                V.tensor_scalar(out=flipg[0:1, 64:72], in0=flipg[0:1, 48:56],
                                scalar1=30.0, scalar2=None, op0=ALU.mult)
                V.tensor_tensor(out=flipg[0:1, 32:40], in0=idxf[:],
                                in1=flipg[0:1, 64:72], op=ALU.subtract)
                V.transpose(fliph[:], flipg[:])
                v8col = fliph[0:8, 0:1]
                ccol = fliph[0:8, 32:33]
                # OH2[w,k] = (30w == idx_k - c'_k)
                P.matmul(ps_a[:, 16:24], ct["c_ones8"][:], flipg[0:1, 64:72],
                         start=True, stop=True)
                V.tensor_tensor(out=oh2s[:], in0=ps_a[:, 16:24],
                                in1=ct["c_i30col"][:].to_broadcast([8, 8]),
                                op=ALU.is_equal)
                # gather: scalar cols then prefixes
                P.matmul(ps_d[:, 10:16], oh2s[:], stb[:, 0:6], start=True, stop=True)
                P.matmul(ps_d[:, 16:16 + T], oh2s[:], stb[:, C_PREF:C_GEND],
                         start=True, stop=True)
                g0 = 10
                # postprocess
                V.tensor_scalar(out=isxu[:], in0=ccol, scalar1=1.0,
                                scalar2=None, op0=ALU.is_ge)
                V.tensor_scalar(out=isxf[:], in0=ccol, scalar1=1.0,
                                scalar2=None, op0=ALU.is_ge)
                V.tensor_scalar(out=data3[:, 0:1], in0=ccol, scalar1=1.0,
                                scalar2=None, op0=ALU.subtract)
                A.copy(out=data3[:, 1:2], in_=ps_d[:, g0 + C_N:g0 + C_N + 1])
                A.copy(out=data3[:, 2:3], in_=ct["c_fresh"][:, t:t + 1])
                V.tensor_copy(stb[:, 0:6], ps_d[:, g0:g0 + 6])
                V.copy_predicated(stb[:, C_L:C_L + 3],
                                  AP(isxu[:].tensor, isxu[:].offset,
                                     [[isxu[:].ap[0][0], 8], [0, 3]]),
                                  data3[:])
                V.copy_predicated(stb[:, C_PB:C_PB + 1], isxu[:], ct["c_zero8"][:])
                V.copy_predicated(stb[:, C_PNB:C_PNB + 1], isxu[:], v8col)
                # newlen
                V.tensor_tensor(out=stb[:, C_LEN:C_LEN + 1],
                                in0=ps_d[:, g0 + C_LEN:g0 + C_LEN + 1],
                                in1=isxf[:], op=ALU.add)
                # prefix: copy gathered, then write char where (pos+1==len)&ext
                A.copy(out=stb[:, C_PREF:C_GEND], in_=ps_d[:, 16:16 + T])
                V.tensor_scalar(out=wmasku[:], in0=ct["c_iota128p1b8"][:],
                                scalar1=stb[:, C_LEN:C_LEN + 1], scalar2=isxf[:],
                                op0=ALU.is_equal, op1=ALU.mult)
                V.copy_predicated(stb[:, C_PREF:C_GEND], wmasku[:],
                                  stb[:, C_L:C_L + 1].to_broadcast([8, T]))

            # ---- finalize ----
            V.tensor_tensor(out=stb[:, C_TOT:C_TOT + 1], in0=stb[:, C_PB:C_PB + 1],
                            in1=stb[:, C_PNB:C_PNB + 1], op=ALU.add)
            A.copy(out=trace[0:1, T:T + 1], in_=stb[0:1, C_TOT:C_TOT + 1])
            V.tensor_scalar(out=lmask[:], in0=ct["c_iota129row"][:],
                            scalar1=lenf[:], scalar2=None, op0=ALU.is_equal)
            V.tensor_tensor(out=bpp[:], in0=trace[:], in1=lmask[:], op=ALU.mult)
            V.tensor_reduce(out=bpp[0:1, 0:1], in_=bpp[:], axis=AXL.X, op=ALU.add)
            nc.gpsimd.dma_start(out=o_bp[:], in_=bpp[0:1, 0:1])
            nc.gpsimd.dma_start(out=o_pref[:], in_=stb[0:1, C_PREF:C_GEND])
            nc.gpsimd.dma_start(out=o_len[:], in_=stb[0:1, C_LEN:C_LEN + 1])

    nc.compile()
    return nc


def _get_nc():
    global _NC
    if _NC is None:
        _NC = _build(T)
    return _NC


def kernel(x, lengths):
    x = np.asarray(x, np.float32)
    lengths = np.asarray(lengths, np.int32)
    B = x.shape[1]
    nc = _get_nc()
    in_maps = []
    for b in range(B):
        m = {k: v for k, v in _CONSTS.items()}
        m["x"] = np.ascontiguousarray(x[:, b, :])
        m["lenf"] = np.array([[float(lengths[b])]], np.float32)
        in_maps.append(m)
    res = run_bass_kernel_spmd(nc, in_maps, list(range(B)))
    dec = np.zeros((B, T), np.int32)
    dl = np.zeros(B, np.int32)
    bp = np.zeros(B, np.float32)
    for b in range(B):
        r = res.results[b]
        dec[b] = np.rint(r["o_pref"][0]).astype(np.int32)
        dl[b] = np.int32(np.rint(r["o_len"][0, 0]))
        bp[b] = r["o_bp"][0, 0]
    return dec, dl, bp


# revision 15
# speedup vs baseline: 1.3701x; 1.0553x over previous
"""CTC prefix beam search decoder on Trainium2 (Bass/Tile).

Data-parallel: one utterance per NeuronCore (batch 8 across 8 cores).
Per core: T=128 sequential DP steps, beam width 8, 29 symbols, fully
unrolled. Merge detection uses exact trie-node ids (small ints in f32)
instead of the reference's uint32 rolling hash — provably the same merge
pattern absent hash collisions. XLA-CPU flush-to-zero is emulated at the
three product sites so the subnormal-band trajectory matches the
reference bit-for-bit.

Self-contained: hardcodes shapes [128, 8, 29] / lengths [8].
"""
import sys

import numpy as np

sys.path.insert(0, "/opt/trn_rl_repo")

from concourse import bacc, mybir  # noqa: E402
from concourse.bass import AP  # noqa: E402
from concourse.tile import TileContext  # noqa: E402
from concourse.bass_utils import run_bass_kernel_spmd  # noqa: E402

f32 = mybir.dt.float32
u32 = mybir.dt.uint32
ALU = mybir.AluOpType
AXL = mybir.AxisListType

T = 128
W = 8
S = 29
BLANK = 28
THRESH = 1e-3
FLT_MIN = 1.1754943508222875e-38
NCAND = W * (S + 1)  # 240, flat order j = w*30 + c' (c'=0 kept, c'=c+1 ext)

# STBIG column map. Pb/Pb_same share col 0 (Pb dies before Pb_same is
# written each step); Pnb/mPnb_kept share col 1.
C_PB = 0
C_PNB = 1
C_L = 2
C_P = 3
C_N = 4
C_LEN = 5
C_PREF = 6      # [., 6:134]
C_GEND = 134
C_TOT = 136
C_PLAST = 137
C_PNBS = C_PNB  # Pnb_same overwrites dead Pnb state
STW = 160


def _consts():
    c = {}
    c["c_ones8"] = np.ones((1, 8), np.float32)
    c["c_ones128"] = np.ones((1, 128), np.float32)
    c["c_ones8col"] = np.ones((8, 1), np.float32)
    c["c_iota29b8"] = np.broadcast_to(np.arange(S, dtype=np.float32), (8, S)).copy()
    c["c_iota128p1b8"] = np.broadcast_to(
        np.arange(1, T + 1, dtype=np.float32), (8, T)).copy()
    c["c_iota128col"] = np.arange(T, dtype=np.float32).reshape(T, 1)
    c["c_iota129row"] = np.arange(T + 1, dtype=np.float32).reshape(1, T + 1)
    c["c_i30col"] = (30.0 * np.arange(W, dtype=np.float32)).reshape(W, 1)
    c["c_i30p30col"] = (30.0 * np.arange(W, dtype=np.float32) + 30.0).reshape(W, 1)
    c["c_ident8"] = np.eye(W, dtype=np.float32)
    fresh = (8.0 + 8.0 * np.arange(T)[None, :] + np.arange(W)[:, None]).astype(
        np.float32)
    c["c_fresh"] = fresh  # [8, 128]
    c["c_m1col"] = np.full((8, 1), -1.0, np.float32)
    c["c_zero8"] = np.zeros((8, 1), np.float32)
    st = np.zeros((8, STW), np.float32)
    st[:, C_L] = -2.0
    st[0, C_L] = -1.0
    st[:, C_P] = -100.0
    st[:, C_N] = -(np.arange(W) + 2.0)
    st[0, C_N] = 0.0
    st[:, C_PREF:C_GEND] = -1.0
    st[0, C_PB] = 1.0
    c["c_stinit"] = st
    return c


_CONSTS = _consts()
_NC = None


def _build(t_steps=T):
    nc = bacc.Bacc("TRN2", target_bir_lowering=False, debug=False, num_devices=8)
    x_in = nc.declare_dram_parameter("x", [T, S], f32, isOutput=False)
    len_in = nc.declare_dram_parameter("lenf", [1, 1], f32, isOutput=False)
    cin = {}
    for k, v in _CONSTS.items():
        cin[k] = nc.declare_dram_parameter(k, list(v.shape), f32, isOutput=False)
    o_pref = nc.declare_dram_parameter("o_pref", [1, T], f32, isOutput=True)
    o_len = nc.declare_dram_parameter("o_len", [1, 1], f32, isOutput=True)
    o_bp = nc.declare_dram_parameter("o_bp", [1, 1], f32, isOutput=True)

    with TileContext(nc) as tc:
        with tc.tile_pool(name="sb", bufs=1) as pool, \
             tc.tile_pool(name="ps", bufs=1, space="PSUM") as psum:
            ct = {k: pool.tile(list(v.shape), f32, tag=k, name=k)
                  for k, v in _CONSTS.items()}
            stb = pool.tile([8, STW], f32, tag="stb")
            x128 = pool.tile([T, S], f32, tag="x128")
            lenf = pool.tile([1, 1], f32, tag="lenf")
            pgt = pool.tile([T, S], f32, tag="pgt")
            pma = pool.tile([T, S], f32, tag="pma")
            pmb_ = pool.tile([T, S], f32, tag="pmb_")
            tmaskf = pool.tile([T, 1], f32, tag="tmaskf")
            flatpm = pool.tile([1, T * S], f32, tag="flatpm")
            pmnb8 = pool.tile([8, T * S], f32, tag="pmnb8")
            pmb8 = pool.tile([8, T], f32, tag="pmb8")
            trace = pool.tile([1, T + 1], f32, tag="trace")
            sc30 = pool.tile([8, 30], f32, tag="sc30")
            flat = pool.tile([1, NCAND], f32, tag="flat")
            idx8 = pool.tile([1, 8], u32, tag="idx8")
            idxf = pool.tile([1, 8], f32, tag="idxf")
            gu = pool.tile([8, S], u32, tag="gu")
            mexu = pool.tile([8, S], u32, tag="mexu")
            base = pool.tile([8, S], f32, tag="base")
            plp = pool.tile([8, S], f32, tag="plp")
            ebuf = pool.tile([8, S], f32, tag="ebuf")
            fme = pool.tile([8, S], f32, tag="fme")
            # flip scratches: a/b: p|n columns->rows ([32,64], n in 2nd block);
            # c/d: E and its transpose; e/f: G and its transpose
            flipa = pool.tile([32, 64], f32, tag="flipa")
            flipb = pool.tile([32, 64], f32, tag="flipb")
            flipc = pool.tile([32, 32], f32, tag="flipc")
            flipd = pool.tile([32, 32], f32, tag="flipd")
            flipe = pool.tile([32, 32], f32, tag="flipe")
            flipf = pool.tile([32, 32], f32, tag="flipf")
            flipg = pool.tile([32, 96], f32, tag="flipg")
            fliph = pool.tile([32, 96], f32, tag="fliph")
            m1t = pool.tile([8, 8], f32, tag="m1t")
            m1 = pool.tile([8, 8], f32, tag="m1")
            mh = pool.tile([8, 8], f32, tag="mh")
            oh2s = pool.tile([8, 8], f32, tag="oh2s")
            data3 = pool.tile([8, 3], f32, tag="data3")
            ft2 = pool.tile([8, 2], f32, tag="ft2")
            fm2x = pool.tile([8, 2], f32, tag="fm2x")
            oh2a = pool.tile([8, 8], f32, tag="oh2a")
            oh2b = pool.tile([8, 8], f32, tag="oh2b")
            ccol = pool.tile([8, 1], f32, tag="ccol")
            wmaskf = pool.tile([8, T], f32, tag="wmaskf")
            diagc = pool.tile([8, 8], f32, tag="diagc")
            mask30 = pool.tile([8, 30], u32, tag="mask30")
            flatm = pool.tile([1, NCAND], u32, tag="flatm")
            isxu = pool.tile([8, 1], u32, tag="isxu")
            isxf = pool.tile([8, 1], f32, tag="isxf")
            wmasku = pool.tile([8, T], u32, tag="wmasku")
            lenb_s = pool.tile([T, 1], f32, tag="lenb_s")
            lmask = pool.tile([1, T + 1], f32, tag="lmask")
            bpp = pool.tile([1, T + 1], f32, tag="bpp")

            ps_a = psum.tile([8, 24], f32, tag="ps_a")     # PB8 | NB8 | b30
            ps_b = psum.tile([8, 40], f32, tag="ps_b")     # H 0:8 | ABS 8:9 | MEX 9:38
            ps_d = psum.tile([8, 16], f32, tag="ps_d")     # gathered scalar cols
            ps_e = psum.tile([8, T], f32, tag="ps_e")      # gathered prefixes
            ps_i = psum.tile([8, 464], f32, tag="ps_i")    # init builds
            ps_l = psum.tile([T, 1], f32, tag="ps_l")      # len bcast

            V, A, P, SY = nc.vector, nc.scalar, nc.tensor, nc.sync
            gf = flipe[0:8, 0:S]     # G lives in the flip scratch
            ehome = flipc[0:8, 0:S]  # E lives in the flip scratch

            # ---- init ----
            for k in _CONSTS:
                nc.gpsimd.dma_start(out=ct[k][:], in_=cin[k][:])
            nc.gpsimd.dma_start(out=stb[:], in_=cin["c_stinit"][:])
            nc.gpsimd.dma_start(out=x128[:], in_=x_in[:])
            nc.gpsimd.dma_start(out=lenf[:], in_=len_in[:])
            for f in (flipa, flipb, flipc, flipd, flipe, flipf, flipg, fliph):
                V.memset(f[:], 0.0)
            V.memset(trace[:], 0.0)

            P.matmul(ps_l[:], ct["c_ones128"][:], lenf[:], start=True, stop=True)
            A.copy(out=lenb_s[:], in_=ps_l[:])
            V.tensor_scalar(out=tmaskf[:], in0=ct["c_iota128col"][:],
                            scalar1=lenb_s[:], scalar2=None, op0=ALU.is_lt)
            V.tensor_scalar(out=pgt[:], in0=x128[:], scalar1=float(THRESH),
                            scalar2=None, op0=ALU.is_gt)
            V.tensor_tensor(out=pma[:], in0=x128[:], in1=pgt[:], op=ALU.mult)
            V.tensor_scalar(out=pmb_[:], in0=pma[:], scalar1=tmaskf[:],
                            scalar2=None, op0=ALU.mult)
            psrc = pmb_[:]
            src = AP(psrc.tensor, psrc.offset, [[psrc.ap[0][0], T], [1, S]])
            pdst = flatpm[:]
            dst = AP(pdst.tensor, pdst.offset, [[pdst.ap[0][0], 1], [1, T * S]])
            SY.dma_start(out=dst, in_=src)
            for i in range(8):
                sl = flatpm[0:1, i * 464:(i + 1) * 464]
                P.matmul(ps_i[:, 0:464], ct["c_ones8"][:], sl, start=True, stop=True)
                A.copy(out=pmnb8[:, i * 464:(i + 1) * 464], in_=ps_i[:, 0:464])
            fsrc = flatpm[:]
            blank_ap = AP(fsrc.tensor, fsrc.offset + BLANK, [[fsrc.ap[0][0], 1], [S, T]])
            P.matmul(ps_i[:, 0:T], ct["c_ones8"][:], blank_ap, start=True, stop=True)
            A.copy(out=pmb8[:], in_=ps_i[:, 0:T])
            p8 = pmnb8[:]
            nb_blank = AP(p8.tensor, p8.offset + BLANK, [[p8.ap[0][0], 8], [S, T]])
            V.memset(nb_blank, 0.0)

            # ---- DP loop ----
            V.memset(mask30[:, 0:1], 0)
            for t in range(t_steps):
                pmnb_t = pmnb8[:, t * S:(t + 1) * S]
                # MEX head first so PE/compare work overlaps the E chain
                fa = flipa[:]
                pn_dst = AP(fa.tensor, fa.offset, [[fa.ap[0][0], 8], [32, 2]])
                V.tensor_copy(pn_dst, stb[:, C_P:C_P + 2])
                V.transpose(flipb[:], flipa[:])
                P.matmul(ps_a[:, 0:8], ct["c_ones8"][:], flipb[0:1, 0:8],
                         start=True, stop=True)
                P.matmul(ps_a[:, 8:16], ct["c_ones8"][:], flipb[0:1, 32:40],
                         start=True, stop=True)
                V.tensor_tensor(out=m1t[:], in0=ps_a[:, 0:8],
                                in1=stb[:, C_N:C_N + 1].to_broadcast([8, 8]),
                                op=ALU.is_equal)
                V.tensor_tensor(out=m1[:], in0=ps_a[:, 8:16],
                                in1=stb[:, C_P:C_P + 1].to_broadcast([8, 8]),
                                op=ALU.is_equal)
                # tot = Pb + Pnb
                V.tensor_tensor(out=stb[:, C_TOT:C_TOT + 1], in0=stb[:, C_PB:C_PB + 1],
                                in1=stb[:, C_PNB:C_PNB + 1], op=ALU.add)
                A.copy(out=trace[0:1, t:t + 1], in_=stb[0:1, C_TOT:C_TOT + 1])
                # G
                V.tensor_tensor(out=gf, in0=ct["c_iota29b8"][:],
                                in1=stb[:, C_L:C_L + 1].to_broadcast([8, S]),
                                op=ALU.is_equal)
                V.tensor_tensor(out=gu[:], in0=ct["c_iota29b8"][:],
                                in1=stb[:, C_L:C_L + 1].to_broadcast([8, S]),
                                op=ALU.is_equal)
                # MEX early: mask goes into mask30 (flat-predicated after DMA)
                P.matmul(ps_b[:, 9:38], m1[:], gf, start=True, stop=True)
                V.tensor_scalar(out=mask30[:, 1:30], in0=ps_b[:, 9:38], scalar1=0.0,
                                scalar2=None, op0=ALU.is_gt)
                # base = select(G, Pb, tot)
                V.tensor_copy(base[:], stb[:, C_TOT:C_TOT + 1].to_broadcast([8, S]))
                V.copy_predicated(base[:], gu[:],
                                  stb[:, C_PB:C_PB + 1].to_broadcast([8, S]))
                # p_last
                V.tensor_tensor(out=plp[:], in0=gf, in1=pmnb_t, op=ALU.mult)
                V.tensor_reduce(out=stb[:, C_PLAST:C_PLAST + 1], in_=plp[:],
                                axis=AXL.X, op=ALU.add)
                # paired FTZ: col0 <- ftz(pmb*tot), col1 <- ftz(p_last*Pnb)
                V.tensor_tensor(out=ft2[:, 0:1], in0=stb[:, C_TOT:C_TOT + 1],
                                in1=pmb8[:, t:t + 1], op=ALU.mult)
                V.tensor_tensor(out=ft2[:, 1:2], in0=stb[:, C_PLAST:C_PLAST + 1],
                                in1=stb[:, C_PNB:C_PNB + 1], op=ALU.mult)
                V.tensor_scalar(out=fm2x[:], in0=ft2[:], scalar1=FLT_MIN,
                                scalar2=None, op0=ALU.is_ge)
                V.tensor_tensor(out=stb[:, 0:2], in0=ft2[:], in1=fm2x[:],
                                op=ALU.mult)
                # E = ftz(pmnb * base) -> flipc
                V.tensor_tensor(out=ebuf[:], in0=pmnb_t, in1=base[:], op=ALU.mult)
                V.tensor_scalar(out=fme[:], in0=ebuf[:], scalar1=FLT_MIN,
                                scalar2=None, op0=ALU.is_ge)
                V.tensor_tensor(out=ehome, in0=ebuf[:], in1=fme[:], op=ALU.mult)
                # E^T, G^T; H; absorbed
                V.transpose(flipd[:], flipc[:])
                V.transpose(flipf[:], flipe[:])
                P.matmul(ps_b[:, 0:8], flipd[0:S, 0:8], flipf[0:S, 0:8],
                         start=True, stop=True)
                V.tensor_tensor(out=mh[:], in0=m1t[:], in1=ps_b[:, 0:8], op=ALU.mult)
                P.matmul(ps_b[:, 8:9], mh[:], ct["c_ones8col"][:],
                         start=True, stop=True)
                # mPnb_kept (in place on col1) ; sc_kept -> sc30 col0
                V.tensor_tensor(out=stb[:, C_PNB:C_PNB + 1],
                                in0=stb[:, C_PNBS:C_PNBS + 1], in1=ps_b[:, 8:9],
                                op=ALU.add)
                V.tensor_tensor(out=sc30[:, 0:1], in0=stb[:, C_PB:C_PB + 1],
                                in1=stb[:, C_PNB:C_PNB + 1], op=ALU.add)
                A.copy(out=sc30[:, 1:30], in_=ehome)
                # flatten scores and mask in parallel; predicate on the flat row
                s30 = sc30[:]
                fsrc30 = AP(s30.tensor, s30.offset, [[s30.ap[0][0], 8], [1, 30]])
                fl = flat[:]
                fdst30 = AP(fl.tensor, fl.offset, [[fl.ap[0][0], 1], [1, NCAND]])
                m30 = mask30[:]
                msrc30 = AP(m30.tensor, m30.offset, [[m30.ap[0][0], 8], [1, 30]])
                fm = flatm[:]
                mdst30 = AP(fm.tensor, fm.offset, [[fm.ap[0][0], 1], [1, NCAND]])
                SY.dma_start(out=mdst30, in_=msrc30)
                SY.dma_start(out=fdst30, in_=fsrc30)
                V.copy_predicated(flat[:], flatm[:],
                                  ct["c_m1col"][0:1, 0:1].to_broadcast([1, NCAND]))
                V.max(flipg[0:1, 0:8], flat[:])
                V.max_index(idx8[:], flipg[0:1, 0:8], flat[:])
                # idx -> flipg row (next to v8) for the shared transpose
                V.tensor_copy(flipg[0:1, 32:40], idx8[:])
                V.transpose(fliph[:], flipg[:])
                v8col = fliph[0:8, 0:1]
                idxcol = fliph[0:8, 32:33]
                # OH2[w,k] = (30w <= idx_k < 30w+30)
                P.matmul(ps_a[:, 16:24], ct["c_ones8"][:], flipg[0:1, 32:40],
                         start=True, stop=True)
                V.tensor_tensor(out=oh2a[:], in0=ps_a[:, 16:24],
                                in1=ct["c_i30col"][:].to_broadcast([8, 8]),
                                op=ALU.is_ge)
                V.tensor_tensor(out=oh2b[:], in0=ps_a[:, 16:24],
                                in1=ct["c_i30p30col"][:].to_broadcast([8, 8]),
                                op=ALU.is_lt)
                V.tensor_tensor(out=oh2s[:], in0=oh2a[:], in1=oh2b[:], op=ALU.mult)
                # gather: scalar cols, prefixes, and 30w column
                P.matmul(ps_d[:, 10:16], oh2s[:], stb[:, 0:6], start=True, stop=True)
                P.matmul(ps_b[:, 38:39], oh2s[:], ct["c_i30col"][:],
                         start=True, stop=True)
                P.matmul(ps_e[:], oh2s[:], stb[:, C_PREF:C_GEND],
                         start=True, stop=False)
                g0 = 10
                # c' column; isext; ec
                V.tensor_tensor(out=ccol[:], in0=idxcol, in1=ps_b[:, 38:39],
                                op=ALU.subtract)
                V.tensor_scalar(out=isxu[:], in0=ccol[:], scalar1=1.0,
                                scalar2=None, op0=ALU.is_ge)
                V.tensor_scalar(out=isxf[:], in0=ccol[:], scalar1=1.0,
                                scalar2=None, op0=ALU.is_ge)
                V.tensor_scalar(out=data3[:, 0:1], in0=ccol[:], scalar1=1.0,
                                scalar2=None, op0=ALU.subtract)
                A.copy(out=data3[:, 1:2], in_=ps_d[:, g0 + C_N:g0 + C_N + 1])
                A.copy(out=data3[:, 2:3], in_=ct["c_fresh"][:, t:t + 1])
                V.tensor_copy(stb[:, 0:6], ps_d[:, g0:g0 + 6])
                V.copy_predicated(stb[:, C_L:C_L + 3],
                                  AP(isxu[:].tensor, isxu[:].offset,
                                     [[isxu[:].ap[0][0], 8], [0, 3]]),
                                  data3[:])
                V.copy_predicated(stb[:, C_PB:C_PB + 1], isxu[:], ct["c_zero8"][:])
                V.copy_predicated(stb[:, C_PNB:C_PNB + 1], isxu[:], v8col)
                # newlen
                V.tensor_tensor(out=stb[:, C_LEN:C_LEN + 1],
                                in0=ps_d[:, g0 + C_LEN:g0 + C_LEN + 1],
                                in1=isxf[:], op=ALU.add)
                # prefix append folded into a second accumulating matmul:
                # psum_pref += diag(c') @ (pos+1 == newlen); kept rows have c'=0
                V.tensor_tensor(out=wmaskf[:], in0=ct["c_iota128p1b8"][:],
                                in1=stb[:, C_LEN:C_LEN + 1].to_broadcast([8, T]),
                                op=ALU.is_equal)
                V.tensor_tensor(out=diagc[:], in0=ct["c_ident8"][:],
                                in1=ccol[:].to_broadcast([8, 8]), op=ALU.mult)
                P.matmul(ps_e[:], diagc[:], wmaskf[:],
                         start=False, stop=True)
                A.copy(out=stb[:, C_PREF:C_GEND], in_=ps_e[:])

            # ---- finalize ----
            V.tensor_tensor(out=stb[:, C_TOT:C_TOT + 1], in0=stb[:, C_PB:C_PB + 1],
                            in1=stb[:, C_PNB:C_PNB + 1], op=ALU.add)
            A.copy(out=trace[0:1, T:T + 1], in_=stb[0:1, C_TOT:C_TOT + 1])
            V.tensor_scalar(out=lmask[:], in0=ct["c_iota129row"][:],
                            scalar1=lenf[:], scalar2=None, op0=ALU.is_equal)
            V.tensor_tensor(out=bpp[:], in0=trace[:], in1=lmask[:], op=ALU.mult)
            V.tensor_reduce(out=bpp[0:1, 0:1], in_=bpp[:], axis=AXL.X, op=ALU.add)
            nc.gpsimd.dma_start(out=o_bp[:], in_=bpp[0:1, 0:1])
            nc.gpsimd.dma_start(out=o_pref[:], in_=stb[0:1, C_PREF:C_GEND])
            nc.gpsimd.dma_start(out=o_len[:], in_=stb[0:1, C_LEN:C_LEN + 1])

    nc.compile()
    return nc


def _get_nc():
    global _NC
    if _NC is None:
        _NC = _build(T)
    return _NC


def kernel(x, lengths):
    x = np.asarray(x, np.float32)
    lengths = np.asarray(lengths, np.int32)
    B = x.shape[1]
    nc = _get_nc()
    in_maps = []
    for b in range(B):
        m = {k: v for k, v in _CONSTS.items()}
        m["x"] = np.ascontiguousarray(x[:, b, :])
        m["lenf"] = np.array([[float(lengths[b])]], np.float32)
        in_maps.append(m)
    res = run_bass_kernel_spmd(nc, in_maps, list(range(B)))
    dec = np.zeros((B, T), np.int32)
    dl = np.zeros(B, np.int32)
    bp = np.zeros(B, np.float32)
    for b in range(B):
        r = res.results[b]
        dec[b] = np.rint(r["o_pref"][0]).astype(np.int32)
        dl[b] = np.int32(np.rint(r["o_len"][0, 0]))
        bp[b] = r["o_bp"][0, 0]
    return dec, dl, bp
